# revision 4
# baseline (speedup 1.0000x reference)
"""Trainium2 Bass kernel for nn_Decoder (sequential LSTMCell scan over provinces).

Computation (per reference):
    for s in range(P):  # P=81 provinces, scan carry h [81, 2048]
        z = h @ W + h_enc[s] @ U + b            # [81, 8192]
        i, f, g, o = split(z); i,f,o = sigmoid  # g identity
        h = o * (f * h + i * g)
        orders[s] = pre-update h
Only orders[0..80] are emitted, so 80 update steps are needed.

Strategy: 8-way tensor parallel over the 4H gate dim. Each core holds a
[2048, 1024] slice of W (column order [o|f|i|g], 256 H-cols per gate) resident
in SBUF, computes its h slice [81, 256] per step, and an AllGather of the
transposed state h^T [2048, 81] feeds the next step's matmul. u_proj = h_enc@U+b
is precomputed on device. Matmuls run in float32r (full PE rate, ~1.5e-4 rel).
"""
import os
import sys

sys.path.insert(0, "/opt/trn_rl_repo")

import numpy as np

P = 81
H = 2048
NC = 8
SL = H // NC          # 256 H-cols per core
GC = 4 * SL           # 1024 gate cols per core
KC = H // 128         # 16 contraction chunks
NSTEP = P - 1         # 80 update steps
NFILL = int(os.environ.get("KERNEL_NFILL", "10"))   # PE warm-keeper matmuls per step

_CACHE = {}


def _build():
    import concourse.bacc as bacc
    import concourse.tile as tile
    from concourse import mybir

    f32 = mybir.dt.float32
    f32r = mybir.dt.float32r
    Sg = mybir.ActivationFunctionType.Sigmoid
    RG = [list(range(NC))]

    nc = bacc.Bacc("TRN2", target_bir_lowering=False, debug=False, num_devices=NC)

    W_d = nc.dram_tensor("Wk", [H, GC], f32r, kind="ExternalInput")
    U_d = nc.dram_tensor("Uk", [H, GC], f32r, kind="ExternalInput")
    bk_d = nc.dram_tensor("bk", [1, GC], f32r, kind="ExternalInput")
    hencT_d = nc.dram_tensor("hencT", [H, P], f32r, kind="ExternalInput")
    hdecT_d = nc.dram_tensor("hdecT", [H, P], f32r, kind="ExternalInput")
    hown0_d = nc.dram_tensor("hown0", [P, SL], f32, kind="ExternalInput")
    ones_d = nc.dram_tensor("ones", [1, P], f32r, kind="ExternalInput")
    iden_d = nc.dram_tensor("iden", [P, P], f32, kind="ExternalInput")
    orders_d = nc.dram_tensor("orders", [P, P, SL], f32, kind="ExternalOutput")

    with tile.TileContext(nc) as tc:
        with tc.tile_pool(name="sbp", bufs=1) as sbp, \
             tc.tile_pool(name="psp", bufs=1, space="PSUM") as psp, \
             tc.tile_pool(name="dramp", bufs=1, space="DRAM") as drp:

            ub_dram = drp.tile([P, GC], f32r, name="ub_dram")
            ag_in = [drp.tile([SL, P], f32r, name=f"agin{i}") for i in range(2)]
            ag_out = [drp.tile([H, P], f32r, addr_space="Shared", name=f"agout{s}")
                      for s in range(NSTEP - 1)]

            # ---------------- prologue ----------------
            ones_t = sbp.tile([1, P], f32r, name="ones_t")
            iden_t = sbp.tile([P, P], f32, name="iden_t")
            bk_t = sbp.tile([1, GC], f32r, name="bk_t")
            nc.sync.dma_start(ones_t[:], ones_d.ap()[:, :])
            nc.sync.dma_start(iden_t[:], iden_d.ap()[:, :])
            nc.sync.dma_start(bk_t[:], bk_d.ap()[:, :])

            h_own = sbp.tile([P, SL], f32, name="hown_init", tag="hown", bufs=2)
            nc.sync.dma_start(h_own[:], hown0_d.ap()[:, :])
            # orders[0] = h_dec_init slice
            nc.sync.dma_start(orders_d.ap()[0, :, :], h_own[:])

            # resident W chunks
            W_sb = []
            for c in range(KC):
                w = sbp.tile([128, GC], f32r, name=f"W{c}", tag=f"W{c}")
                nc.sync.dma_start(w[:], W_d.ap()[c * 128:(c + 1) * 128, :])
                W_sb.append(w)

            # initial gathered state = h_dec_init^T (from host)
            hT = []
            for c in range(KC):
                t = sbp.tile([128, P], f32r, name=f"hT_init{c}", tag=f"hT{c}", bufs=2)
                nc.sync.dma_start(t[:], hdecT_d.ap()[c * 128:(c + 1) * 128, :])
                hT.append(t)

            # u_proj + b -> ub_dram  (ub[t] = h_enc[t] @ U_k + b_k)
            ub_ps = psp.tile([P, GC], f32, name="ub_ps", tag="z", bufs=2)
            for n in range(2):
                sl = slice(n * 512, (n + 1) * 512)
                nc.tensor.matmul(ub_ps[:, sl], ones_t[:], bk_t[:, sl],
                                 start=True, stop=False)
            for c in range(KC):
                he = sbp.tile([128, P], f32r, name=f"he{c}", tag="hencT", bufs=4)
                uu = sbp.tile([128, GC], f32r, name=f"uu{c}", tag="U", bufs=4)
                nc.sync.dma_start(he[:], hencT_d.ap()[c * 128:(c + 1) * 128, :])
                nc.sync.dma_start(uu[:], U_d.ap()[c * 128:(c + 1) * 128, :])
                for n in range(2):
                    sl = slice(n * 512, (n + 1) * 512)
                    nc.tensor.matmul(ub_ps[:, sl], he[:], uu[:, sl],
                                     start=False, stop=(c == KC - 1))
            ub_sb = sbp.tile([P, GC], f32r, name="ub_sb")
            nc.vector.tensor_copy(ub_sb[:], ub_ps[:])
            nc.sync.dma_start(ub_dram[:], ub_sb[:])


            # ---------------- steps ----------------
            for s in range(NSTEP):
                ub_row = sbp.tile([1, GC], f32r, name=f"ubr{s}", tag="ubr", bufs=3)
                nc.sync.dma_start(ub_row[:], ub_dram[:][s:s + 1, :])

                z = psp.tile([P, GC], f32, name=f"z{s}", tag="z", bufs=2)
                for n in range(2):
                    sl = slice(n * 512, (n + 1) * 512)
                    nc.tensor.matmul(z[:, sl], ones_t[:], ub_row[:, sl],
                                     start=True, stop=False)
                    for c in range(KC):
                        nc.tensor.matmul(z[:, sl], hT[c][:], W_sb[c][:, sl],
                                         start=False, stop=(c == KC - 1))

                # gates: cols [o f i g], each SL=256
                o_s = sbp.tile([P, SL], f32, name=f"o{s}", tag="o_s", bufs=2)
                f_s = sbp.tile([P, SL], f32, name=f"f{s}", tag="f_s", bufs=2)
                i_s = sbp.tile([P, SL], f32, name=f"i{s}", tag="i_s", bufs=2)
                t1 = sbp.tile([P, SL], f32, name=f"t1_{s}", tag="t1", bufs=2)
                t2 = sbp.tile([P, SL], f32, name=f"t2_{s}", tag="t2", bufs=2)
                c_t = sbp.tile([P, SL], f32, name=f"c{s}", tag="c_t", bufs=2)
                h_n = sbp.tile([P, SL], f32, name=f"h{s + 1}", tag="hown", bufs=2)
                nc.scalar.activation(o_s[:], z[:, 0:SL], Sg)
                nc.scalar.activation(f_s[:], z[:, SL:2 * SL], Sg)
                nc.vector.tensor_mul(t1[:], f_s[:], h_own[:])
                nc.scalar.activation(i_s[:], z[:, 2 * SL:3 * SL], Sg)
                nc.vector.tensor_mul(t2[:], i_s[:], z[:, 3 * SL:4 * SL])
                nc.vector.tensor_add(c_t[:], t1[:], t2[:])
                nc.vector.tensor_mul(h_n[:], o_s[:], c_t[:])

                nc.sync.dma_start(orders_d.ap()[s + 1, :, :], h_n[:])

                if s < NSTEP - 1:
                    pack = sbp.tile([128, 2 * P], f32r, name=f"pk{s}", tag="pack", bufs=2)
                    for c2 in range(2):
                        px = psp.tile([128, P], f32, name=f"px{s}_{c2}",
                                      tag="px", bufs=2)
                        nc.tensor.transpose(px[:], h_n[:, c2 * 128:(c2 + 1) * 128],
                                            iden_t[:])
                        nc.vector.tensor_copy(pack[:, c2 * P:(c2 + 1) * P], px[:])
                    agin = ag_in[s % 2]
                    for c2 in range(2):
                        for hp in range(2):
                            nc.sync.dma_start(
                                agin[:][c2 * 128 + hp * 64:c2 * 128 + (hp + 1) * 64, :],
                                pack[hp * 64:(hp + 1) * 64, c2 * P:(c2 + 1) * P])
                    nc.gpsimd.collective_compute(
                        "AllGather", mybir.AluOpType.bypass, replica_groups=RG,
                        ins=[agin[:]], outs=[ag_out[s][:]])
                    newhT = []
                    for c in range(KC):
                        t = sbp.tile([128, P], f32r, name=f"hT{s}_{c}",
                                     tag=f"hT{c}", bufs=2)
                        nc.sync.dma_start(t[:], ag_out[s][:][c * 128:(c + 1) * 128, :])
                        newhT.append(t)
                    # PE warm-keepers during the AllGather wait
                    for fi in range(NFILL):
                        fps = psp.tile([P, 512], f32, name=f"fl{s}_{fi}",
                                       tag="fill", bufs=1)
                        nc.tensor.matmul(fps[:], ones_t[:], ub_sb[0:1, 0:512],
                                         start=True, stop=True)
                    hT = newhT
                h_own = h_n

    nc.finalize()
    return nc


def _shard_inputs(h_enc, mask, h_dec_init, W, U, b):
    h_enc = np.ascontiguousarray(np.asarray(h_enc, dtype=np.float32))
    h_dec_init = np.ascontiguousarray(np.asarray(h_dec_init, dtype=np.float32))
    W = np.asarray(W, dtype=np.float32)
    U = np.asarray(U, dtype=np.float32)
    b = np.asarray(b, dtype=np.float32)

    hencT = np.ascontiguousarray(h_enc.T)       # [H, P]
    hdecT = np.ascontiguousarray(h_dec_init.T)  # [H, P]
    ones = np.ones((1, P), dtype=np.float32)
    iden = np.eye(P, dtype=np.float32)

    # reference gate order in 4H: [i f g o]; per-core column order [o f i g]
    gate_off = {"i": 0, "f": H, "g": 2 * H, "o": 3 * H}
    order = ["o", "f", "i", "g"]
    in_maps = []
    for k in range(NC):
        cols = np.concatenate(
            [np.arange(gate_off[g] + k * SL, gate_off[g] + (k + 1) * SL)
             for g in order])
        Wk = np.ascontiguousarray(W[:, cols])
        Uk = np.ascontiguousarray(U[:, cols])
        bk = np.ascontiguousarray(b[cols][None, :])
        hown0 = np.ascontiguousarray(h_dec_init[:, k * SL:(k + 1) * SL])
        in_maps.append({
            "Wk": Wk, "Uk": Uk, "bk": bk, "hencT": hencT, "hdecT": hdecT,
            "hown0": hown0, "ones": ones, "iden": iden,
        })
    return in_maps


def _run(inputs, trace=False):
    from concourse import bass_utils

    if "nc" not in _CACHE:
        _CACHE["nc"] = _build()
    nc = _CACHE["nc"]
    in_maps = _shard_inputs(**inputs)
    res = bass_utils.run_bass_kernel_spmd(
        nc, in_maps, core_ids=list(range(NC)), trace=trace)
    out = np.concatenate(
        [res.results[k]["orders"] for k in range(NC)], axis=2)
    return np.ascontiguousarray(out.astype(np.float32)), res


def kernel(h_enc, mask, h_dec_init, W, U, b):
    out, _ = _run(dict(h_enc=h_enc, mask=mask, h_dec_init=h_dec_init,
                       W=W, U=U, b=b), trace=False)
    return out
